# revision 8
# baseline (speedup 1.0000x reference)
"""Causal multi-head attention (B=2, S=2048, D=1024, H=16, Dh=64) on 8 TRN2
NeuronCores.

Sharding: core c handles batch c//4 and heads 4*(c%4) .. 4*(c%4)+3 (data
parallel on batch x tensor parallel on heads). Each core is fully
independent: it gets x[b] and the 256-wide column slices of Wq/Wk/Wv for its
4 heads, and returns its heads' outputs as a [2048, 256] slice; the host
reassembles the full [2, 2048, 1024] output by concatenation.

Device kernel (per core); matmul operands in bf16 (fp32 PSUM accumulate),
softmax normalization in fp32:
  A. x -> xT [1024,2048] via PE transposes (128x128 f32 blocks), cast to
     bf16 in the PSUM->SBUF copy.
  B. QT = Wq^T x^T + bq ([c, s] layout, one head pair per 128-partition
     tile), same for KT; V = x Wv + bv in natural [s, c] layout, stored
     augmented with a ones column per head (V_aug[:, 65h+64] = 1) so the
     attention matmul also produces the softmax denominator.
  C. Per head pair, per 512-wide q chunk, over k tiles up to the diagonal:
     scores^T[k,q] for both heads land in one 2-bank PSUM tile, one Exp
     per k tile on ScalarE (scale=1/8; no max subtraction needed, scores
     are ~N(0,1)), causal handling by column pruning + 128x128
     triangular-mask multiplies on diagonal blocks, then
     O^T[d,q] += V_aug^T @ expS accumulated in PSUM (row 64 = sum of exp).
     Tail per head: copy O^T to SBUF, PE-transpose each 128-q block back to
     [q, 65]; the denominator is then one per partition, so a [128,1]
     reciprocal + tensor_scalar multiply normalizes; DMA out in natural
     [s, e] layout.
"""

import ml_dtypes
import numpy as np

import concourse.bass as bass
import concourse.mybir as mybir
import concourse.tile as tile
from concourse.bass_utils import run_bass_kernel_spmd
from concourse.masks import make_identity, make_upper_triangular

B = 2
S = 2048
D = 1024
H = 16
DH = 64
N_CORES = 8
HPC = 4          # heads per core
CW = HPC * DH    # 256: W column slice width per core
QCH = 512        # q chunk width
F32 = mybir.dt.float32
F32R = mybir.dt.float32r
BF16 = mybir.dt.bfloat16
DT = BF16        # matmul operand dtype
EXP = mybir.ActivationFunctionType.Exp
MULT = mybir.AluOpType.mult
ADD = mybir.AluOpType.add

_STATE = {}


def _split_sync_waits(nc, max_waits=1):
    """This walrus rejects instructions carrying more than ~2 sem-waits
    ("Too many sync wait commands"). Move excess waits emitted by Tile onto
    same-engine NoOps inserted right before the instruction."""
    n = 0
    for f in nc.m.functions:
        for bb in f.blocks:
            il = bb.instructions
            i = 0
            while i < len(il):
                ins = il[i]
                si = getattr(ins, "sync_info", None)
                if si is not None and len(si.on_wait) > max_waits:
                    waits = list(si.on_wait)
                    keep = waits[len(waits) - max_waits:]
                    extra = waits[: len(waits) - max_waits]
                    ins.sync_info = mybir.SyncInfo(
                        on_wait=keep, on_update=list(si.on_update)
                    )
                    pos = i
                    for j in range(0, len(extra), max_waits):
                        nop = mybir.InstNoOp(
                            name=f"{ins.name}-waitsplit{j}",
                            engine=ins.engine,
                            sync_info=mybir.SyncInfo(
                                on_wait=extra[j : j + max_waits], on_update=[]
                            ),
                            bass_nofuse=True,
                        )
                        il.insert(pos, nop)
                        pos += 1
                        i += 1
                    n += 1
                i += 1
    return n


def _build():
    nc = bass.Bass()
    xt_d = nc.dram_tensor("xt", [D, S], BF16, kind="ExternalInput")
    wq_d = nc.dram_tensor("wq", [D, CW], BF16, kind="ExternalInput")
    wk_d = nc.dram_tensor("wk", [D, CW], BF16, kind="ExternalInput")
    wv_d = nc.dram_tensor("wv", [D, CW], BF16, kind="ExternalInput")
    bq_d = nc.dram_tensor("bq", [CW], F32, kind="ExternalInput")
    bk_d = nc.dram_tensor("bk", [CW], F32, kind="ExternalInput")
    bv_d = nc.dram_tensor("bv", [CW], BF16, kind="ExternalInput")
    out_d = nc.dram_tensor("out", [S, CW], F32, kind="ExternalOutput")

    ND = D // 128   # 8 d tiles
    NS = S // 128   # 16 s tiles
    NQ = S // QCH   # 4 q chunks

    with tile.TileContext(nc) as tc:
        with (
            tc.tile_pool(name="const", bufs=1) as cp,
            tc.tile_pool(name="big", bufs=1) as bigp,
        ):
            idf = cp.tile([128, 128], F32, tag="idf")
            tri32 = cp.tile([128, 128], F32, tag="tri32")
            ones32 = cp.tile([128, 128], F32, tag="ones32")
            tri = cp.tile([128, 128], DT, tag="tri")
            make_identity(nc, idf[:])
            make_upper_triangular(nc, tri32[:], val=1.0, diag=True)
            nc.gpsimd.memset(ones32[:], 1.0)
            nc.vector.tensor_copy(tri[:], tri32[:])

            # weights (bf16 via SWDGE cast-DMA) and biases
            wq = [bigp.tile([128, CW], DT, tag=f"wq{k}", name=f"wq{k}") for k in range(ND)]
            wk = [bigp.tile([128, CW], DT, tag=f"wk{k}", name=f"wk{k}") for k in range(ND)]
            wv = [bigp.tile([128, CW], DT, tag=f"wv{k}", name=f"wv{k}") for k in range(ND)]
            for k in range(ND):
                nc.sync.dma_start(out=wq[k][:], in_=wq_d[128 * k : 128 * (k + 1), :])
                nc.sync.dma_start(out=wk[k][:], in_=wk_d[128 * k : 128 * (k + 1), :])
                nc.sync.dma_start(out=wv[k][:], in_=wv_d[128 * k : 128 * (k + 1), :])
            bqs = cp.tile([128, 2], F32, tag="bqs")
            bks = cp.tile([128, 2], F32, tag="bks")
            bvr = cp.tile([1, CW], DT, tag="bvr")
            nc.sync.dma_start(out=bqs[:], in_=bq_d.rearrange("(t p) -> p t", p=128))
            nc.sync.dma_start(out=bks[:], in_=bk_d.rearrange("(t p) -> p t", p=128))
            nc.sync.dma_start(out=bvr[:], in_=bv_d[None, :])
            onesb = cp.tile([1, 128], DT, tag="onesb")
            nc.vector.tensor_copy(onesb[:], ones32[0:1, :])
            ones_d = cp.tile([128, HPC], DT, tag="ones_d")
            nc.vector.tensor_copy(ones_d[:], ones32[:, 0:HPC])

            xT = [bigp.tile([128, S], DT, tag=f"xT{k}", name=f"xT{k}") for k in range(ND)]
            qt = [bigp.tile([128, S], DT, tag=f"qt{t}", name=f"qt{t}") for t in range(2)]
            kt = [bigp.tile([128, S], DT, tag=f"kt{t}", name=f"kt{t}") for t in range(2)]
            va = [bigp.tile([128, 65 * HPC], DT, tag=f"va{i}", name=f"va{i}") for i in range(NS)]

            # Phase A: xT comes pre-transposed in bf16 from the host.
            # Chunked along s so phase B's first matmuls start early.
            for j in range(NQ):
                for k in range(ND):
                    nc.sync.dma_start(
                        out=xT[k][:, QCH * j : QCH * (j + 1)],
                        in_=xt_d[128 * k : 128 * (k + 1), QCH * j : QCH * (j + 1)],
                    )

            # Phase B: projections
            with tc.tile_pool(name="pp", bufs=4, space="PSUM") as pp:
                for w, dstT, bsl in ((wq, qt, bqs), (wk, kt, bks)):
                    for t in range(2):
                        for j in range(NQ):
                            ppt = pp.tile([128, QCH], F32, tag="ppt")
                            for k in range(ND):
                                nc.tensor.matmul(
                                    ppt[:],
                                    w[k][:, 128 * t : 128 * (t + 1)],
                                    xT[k][:, QCH * j : QCH * (j + 1)],
                                    start=(k == 0),
                                    stop=(k == ND - 1),
                                )
                            nc.vector.tensor_scalar_add(
                                dstT[t][:, QCH * j : QCH * (j + 1)], ppt[:], bsl[:, t : t + 1]
                            )
                # bias broadcast tile for V
                ppb = pp.tile([128, CW], F32, tag="ppt")
                nc.tensor.matmul(ppb[:], onesb[0:1, :], bvr[0:1, :], start=True, stop=True)
                bcv = cp.tile([128, CW], F32, tag="bcv")
                nc.vector.tensor_copy(bcv[:], ppb[:])
                for i in range(NS):
                    ppv = pp.tile([128, CW], F32, tag="ppt")
                    for k in range(ND):
                        nc.tensor.matmul(
                            ppv[:],
                            xT[k][:, 128 * i : 128 * (i + 1)],
                            wv[k][:],
                            start=(k == 0),
                            stop=(k == ND - 1),
                        )
                    # scatter per-head 64 cols into 65-strided layout, adding bias
                    nc.vector.tensor_tensor(
                        out=va[i].rearrange("p (h e) -> p h e", h=HPC)[:, :, 0:DH],
                        in0=ppv.rearrange("p (h e) -> p h e", e=DH),
                        in1=bcv.rearrange("p (h e) -> p h e", e=DH),
                        op=ADD,
                    )
                    # ones column per head (col 65h+64)
                    nc.vector.tensor_copy(
                        va[i].rearrange("p (h e) -> p h e", h=HPC)[:, :, DH : DH + 1],
                        ones_d[:, :, None],
                    )

            # Phase C: attention; both heads of a pair share one 2-bank psS tile
            with (
                tc.tile_pool(name="esp", bufs=6) as esp,
                tc.tile_pool(name="otp", bufs=3) as otp,
                tc.tile_pool(name="tlp", bufs=4) as tlp,
                tc.tile_pool(name="psc", bufs=3, space="PSUM") as psc,
                tc.tile_pool(name="pso", bufs=2, space="PSUM") as pso,
            ):
                for t in range(2):          # head pair tile
                    for j in range(NQ):     # q chunk
                        q0 = QCH * j
                        last = 4 * j + 3
                        po = [
                            pso.tile([128, QCH], F32, tag="po", name=f"po{t}{j}{h}")
                            for h in range(2)
                        ]
                        for tt in range(4 * j + 4):   # k tiles
                            r = max(0, 128 * tt - q0)
                            pss = psc.tile([128, 2 * QCH], F32, tag="pss")
                            for h, base in ((0, 0), (1, 64)):
                                nc.tensor.matmul(
                                    pss[:, h * QCH + r : (h + 1) * QCH],
                                    kt[t][base : base + 64, 128 * tt : 128 * (tt + 1)],
                                    qt[t][base : base + 64, q0 + r : q0 + QCH],
                                    start=True,
                                    stop=True,
                                    tile_position=(base, 0),
                                )
                            es = esp.tile([128, 2 * QCH], DT, tag="es")
                            if r == 0:
                                nc.scalar.activation(es[:], pss[:], EXP, scale=0.125)
                            else:
                                w3 = QCH - r
                                nc.scalar.activation(
                                    es.rearrange("p (h q) -> p h q", h=2)[:, :, r:QCH],
                                    pss.rearrange("p (h q) -> p h q", h=2)[:, :, r:QCH],
                                    EXP,
                                    scale=0.125,
                                )
                            if tt >= 4 * j:  # diagonal block: triangular mask
                                for h in range(2):
                                    nc.vector.tensor_tensor(
                                        out=es[:, h * QCH + r : h * QCH + r + 128],
                                        in0=es[:, h * QCH + r : h * QCH + r + 128],
                                        in1=tri[:],
                                        op=MULT,
                                    )
                            for h in range(2):
                                hcol = 65 * (2 * t + h)
                                nc.tensor.matmul(
                                    po[h][0:65, r:QCH],
                                    va[tt][:, hcol : hcol + 65],
                                    es[:, h * QCH + r : (h + 1) * QCH],
                                    start=(tt == 0),
                                    stop=(tt == last),
                                )
                        # tails: transpose back, per-partition reciprocal, store
                        for h in range(2):
                            hl = 2 * t + h
                            ot = otp.tile([128, QCH], F32, tag="ot")
                            nc.vector.tensor_copy(ot[0:65, :], po[h][0:65, :])
                            for c in range(QCH // 128):
                                pot = pso.tile([128, 65], F32, tag="po", name=f"pot{t}{j}{h}{c}")
                                nc.tensor.transpose(
                                    pot[:], ot[0:65, 128 * c : 128 * (c + 1)], idf[0:65, 0:65]
                                )
                                rc = tlp.tile([128, 1], F32, tag="rc")
                                nc.vector.reciprocal(rc[:], pot[:, 64:65])
                                on = tlp.tile([128, DH], F32, tag="on")
                                nc.vector.tensor_scalar_mul(on[:], pot[:, 0:DH], rc[:])
                                nc.sync.dma_start(
                                    out=out_d[
                                        q0 + 128 * c : q0 + 128 * (c + 1),
                                        hl * DH : (hl + 1) * DH,
                                    ],
                                    in_=on[:],
                                )

    _split_sync_waits(nc)
    return nc


def _get_nc():
    if "nc" not in _STATE:
        _STATE["nc"] = _build()
    return _STATE["nc"]


def kernel(**inputs):
    x = np.asarray(inputs["x"], dtype=np.float32)
    wq = np.asarray(inputs["Wq"], dtype=np.float32).astype(ml_dtypes.bfloat16)
    wk = np.asarray(inputs["Wk"], dtype=np.float32).astype(ml_dtypes.bfloat16)
    wv = np.asarray(inputs["Wv"], dtype=np.float32).astype(ml_dtypes.bfloat16)
    bq = np.asarray(inputs["bq"], dtype=np.float32)
    bk = np.asarray(inputs["bk"], dtype=np.float32)
    bv = np.asarray(inputs["bv"], dtype=np.float32).astype(ml_dtypes.bfloat16)
    xts = [np.ascontiguousarray(x[b].T).astype(ml_dtypes.bfloat16) for b in range(B)]

    in_maps = []
    for c in range(N_CORES):
        b, hg = divmod(c, HPC)
        sl = slice(CW * hg, CW * (hg + 1))
        in_maps.append(
            {
                "xt": xts[b],
                "wq": np.ascontiguousarray(wq[:, sl]),
                "wk": np.ascontiguousarray(wk[:, sl]),
                "wv": np.ascontiguousarray(wv[:, sl]),
                "bq": np.ascontiguousarray(bq[sl]),
                "bk": np.ascontiguousarray(bk[sl]),
                "bv": np.ascontiguousarray(bv[sl]),
            }
        )

    nc = _get_nc()
    res = run_bass_kernel_spmd(nc, in_maps, list(range(N_CORES)))
    _STATE["last_result"] = res

    out = np.empty((B, S, D), dtype=np.float32)
    for c in range(N_CORES):
        b, hg = divmod(c, HPC)
        out[b, :, CW * hg : CW * (hg + 1)] = res.results[c]["out"]
    return out


# revision 9
# speedup vs baseline: 1.0547x; 1.0547x over previous
"""Causal multi-head attention (B=2, S=2048, D=1024, H=16, Dh=64) on 8 TRN2
NeuronCores.

Sharding: core c handles batch c//4 and heads 4*(c%4) .. 4*(c%4)+3 (data
parallel on batch x tensor parallel on heads). Each core is fully
independent: it gets x[b] and the 256-wide column slices of Wq/Wk/Wv for its
4 heads, and returns its heads' outputs as a [2048, 256] slice; the host
reassembles the full [2, 2048, 1024] output by concatenation.

Device kernel (per core); matmul operands in bf16 (fp32 PSUM accumulate),
softmax normalization in fp32:
  A. x -> xT [1024,2048] via PE transposes (128x128 f32 blocks), cast to
     bf16 in the PSUM->SBUF copy.
  B. QT = Wq^T x^T + bq ([c, s] layout, one head pair per 128-partition
     tile), same for KT; V = x Wv + bv in natural [s, c] layout, stored
     augmented with a ones column per head (V_aug[:, 65h+64] = 1) so the
     attention matmul also produces the softmax denominator.
  C. Per head pair, per 512-wide q chunk, over k tiles up to the diagonal:
     scores^T[k,q] for both heads land in one 2-bank PSUM tile, one Exp
     per k tile on ScalarE (scale=1/8; no max subtraction needed, scores
     are ~N(0,1)), causal handling by column pruning + 128x128
     triangular-mask multiplies on diagonal blocks, then
     O^T[d,q] += V_aug^T @ expS accumulated in PSUM (row 64 = sum of exp).
     Tail per head: copy O^T to SBUF, PE-transpose each 128-q block back to
     [q, 65]; the denominator is then one per partition, so a [128,1]
     reciprocal + tensor_scalar multiply normalizes; DMA out in natural
     [s, e] layout.
"""

import ml_dtypes
import numpy as np

import concourse.bass as bass
import concourse.mybir as mybir
import concourse.tile as tile
from concourse.bass_utils import run_bass_kernel_spmd
from concourse.masks import make_identity, make_upper_triangular

B = 2
S = 2048
D = 1024
H = 16
DH = 64
N_CORES = 8
HPC = 4          # heads per core
CW = HPC * DH    # 256: W column slice width per core
QCH = 512        # q chunk width
F32 = mybir.dt.float32
F32R = mybir.dt.float32r
BF16 = mybir.dt.bfloat16
DT = BF16        # matmul operand dtype
EXP = mybir.ActivationFunctionType.Exp
MULT = mybir.AluOpType.mult
ADD = mybir.AluOpType.add

_STATE = {}


def _split_sync_waits(nc, max_waits=1):
    """This walrus rejects instructions carrying more than ~2 sem-waits
    ("Too many sync wait commands"). Move excess waits emitted by Tile onto
    same-engine NoOps inserted right before the instruction."""
    n = 0
    for f in nc.m.functions:
        for bb in f.blocks:
            il = bb.instructions
            i = 0
            while i < len(il):
                ins = il[i]
                si = getattr(ins, "sync_info", None)
                if si is not None and len(si.on_wait) > max_waits:
                    waits = list(si.on_wait)
                    keep = waits[len(waits) - max_waits:]
                    extra = waits[: len(waits) - max_waits]
                    ins.sync_info = mybir.SyncInfo(
                        on_wait=keep, on_update=list(si.on_update)
                    )
                    pos = i
                    for j in range(0, len(extra), max_waits):
                        nop = mybir.InstNoOp(
                            name=f"{ins.name}-waitsplit{j}",
                            engine=ins.engine,
                            sync_info=mybir.SyncInfo(
                                on_wait=extra[j : j + max_waits], on_update=[]
                            ),
                            bass_nofuse=True,
                        )
                        il.insert(pos, nop)
                        pos += 1
                        i += 1
                    n += 1
                i += 1
    return n


def _build():
    nc = bass.Bass()
    xt_d = nc.dram_tensor("xt", [D, S], BF16, kind="ExternalInput")
    wq_d = nc.dram_tensor("wq", [D, CW], BF16, kind="ExternalInput")
    wk_d = nc.dram_tensor("wk", [D, CW], BF16, kind="ExternalInput")
    wv_d = nc.dram_tensor("wv", [D, CW], BF16, kind="ExternalInput")
    bq_d = nc.dram_tensor("bq", [CW], F32, kind="ExternalInput")
    bk_d = nc.dram_tensor("bk", [CW], F32, kind="ExternalInput")
    bv_d = nc.dram_tensor("bv", [CW], BF16, kind="ExternalInput")
    out_d = nc.dram_tensor("out", [S, CW], F32, kind="ExternalOutput")

    ND = D // 128   # 8 d tiles
    NS = S // 128   # 16 s tiles
    NQ = S // QCH   # 4 q chunks

    with tile.TileContext(nc) as tc:
        with (
            tc.tile_pool(name="const", bufs=1) as cp,
            tc.tile_pool(name="big", bufs=1) as bigp,
        ):
            idf = cp.tile([128, 128], F32, tag="idf")
            tri32 = cp.tile([128, 128], F32, tag="tri32")
            ones32 = cp.tile([128, 128], F32, tag="ones32")
            tri = cp.tile([128, 128], DT, tag="tri")
            make_identity(nc, idf[:])
            make_upper_triangular(nc, tri32[:], val=1.0, diag=True)
            nc.gpsimd.memset(ones32[:], 1.0)
            nc.vector.tensor_copy(tri[:], tri32[:])

            # weights (bf16 via SWDGE cast-DMA) and biases
            wq = [bigp.tile([128, CW], DT, tag=f"wq{k}", name=f"wq{k}") for k in range(ND)]
            wk = [bigp.tile([128, CW], DT, tag=f"wk{k}", name=f"wk{k}") for k in range(ND)]
            wv = [bigp.tile([128, CW], DT, tag=f"wv{k}", name=f"wv{k}") for k in range(ND)]
            for k in range(ND):
                nc.sync.dma_start(out=wq[k][:], in_=wq_d[128 * k : 128 * (k + 1), :])
                nc.sync.dma_start(out=wk[k][:], in_=wk_d[128 * k : 128 * (k + 1), :])
                nc.sync.dma_start(out=wv[k][:], in_=wv_d[128 * k : 128 * (k + 1), :])
            bqs = cp.tile([128, 2], F32, tag="bqs")
            bks = cp.tile([128, 2], F32, tag="bks")
            bvr = cp.tile([1, CW], DT, tag="bvr")
            nc.sync.dma_start(out=bqs[:], in_=bq_d.rearrange("(t p) -> p t", p=128))
            nc.sync.dma_start(out=bks[:], in_=bk_d.rearrange("(t p) -> p t", p=128))
            nc.sync.dma_start(out=bvr[:], in_=bv_d[None, :])
            onesb = cp.tile([1, 128], DT, tag="onesb")
            nc.vector.tensor_copy(onesb[:], ones32[0:1, :])
            ones_d = cp.tile([128, HPC], DT, tag="ones_d")
            nc.vector.tensor_copy(ones_d[:], ones32[:, 0:HPC])

            xT = [bigp.tile([128, S], DT, tag=f"xT{k}", name=f"xT{k}") for k in range(ND)]
            qt = [bigp.tile([128, S], DT, tag=f"qt{t}", name=f"qt{t}") for t in range(2)]
            kt = [bigp.tile([128, S], DT, tag=f"kt{t}", name=f"kt{t}") for t in range(2)]
            va = [bigp.tile([128, 65 * HPC], DT, tag=f"va{i}", name=f"va{i}") for i in range(NS)]

            # Phase A: xT comes pre-transposed in bf16 from the host.
            # Chunked along s so phase B's first matmuls start early.
            for j in range(NQ):
                for k in range(ND):
                    nc.sync.dma_start(
                        out=xT[k][:, QCH * j : QCH * (j + 1)],
                        in_=xt_d[128 * k : 128 * (k + 1), QCH * j : QCH * (j + 1)],
                    )

            # Phase B: projections
            with tc.tile_pool(name="pp", bufs=4, space="PSUM") as pp:
                for w, dstT, bsl in ((wq, qt, bqs), (wk, kt, bks)):
                    for t in range(2):
                        for j in range(NQ):
                            ppt = pp.tile([128, QCH], F32, tag="ppt")
                            for k in range(ND):
                                nc.tensor.matmul(
                                    ppt[:],
                                    w[k][:, 128 * t : 128 * (t + 1)],
                                    xT[k][:, QCH * j : QCH * (j + 1)],
                                    start=(k == 0),
                                    stop=(k == ND - 1),
                                )
                            nc.vector.tensor_scalar_add(
                                dstT[t][:, QCH * j : QCH * (j + 1)], ppt[:], bsl[:, t : t + 1]
                            )
                # bias broadcast tile for V
                ppb = pp.tile([128, CW], F32, tag="ppt")
                nc.tensor.matmul(ppb[:], onesb[0:1, :], bvr[0:1, :], start=True, stop=True)
                bcv = cp.tile([128, CW], F32, tag="bcv")
                nc.vector.tensor_copy(bcv[:], ppb[:])
                for i in range(NS):
                    ppv = pp.tile([128, CW], F32, tag="ppt")
                    for k in range(ND):
                        nc.tensor.matmul(
                            ppv[:],
                            xT[k][:, 128 * i : 128 * (i + 1)],
                            wv[k][:],
                            start=(k == 0),
                            stop=(k == ND - 1),
                        )
                    # scatter per-head 64 cols into 65-strided layout, adding bias
                    nc.vector.tensor_tensor(
                        out=va[i].rearrange("p (h e) -> p h e", h=HPC)[:, :, 0:DH],
                        in0=ppv.rearrange("p (h e) -> p h e", e=DH),
                        in1=bcv.rearrange("p (h e) -> p h e", e=DH),
                        op=ADD,
                    )
                    # ones column per head (col 65h+64)
                    nc.vector.tensor_copy(
                        va[i].rearrange("p (h e) -> p h e", h=HPC)[:, :, DH : DH + 1],
                        ones_d[:, :, None],
                    )

            # Phase C: attention; both heads of a pair share one 2-bank psS tile
            with (
                tc.tile_pool(name="esp", bufs=6) as esp,
                tc.tile_pool(name="otp", bufs=3) as otp,
                tc.tile_pool(name="tlp", bufs=4) as tlp,
                tc.tile_pool(name="psc", bufs=2, space="PSUM") as psc,
                tc.tile_pool(name="pso", bufs=4, space="PSUM") as pso,
            ):
                for t in range(2):          # head pair tile
                    for j in range(NQ):     # q chunk
                        q0 = QCH * j
                        last = 4 * j + 3
                        po = [
                            pso.tile([128, QCH], F32, tag="po", name=f"po{t}{j}{h}")
                            for h in range(2)
                        ]
                        for tt in range(4 * j + 4):   # k tiles
                            r = max(0, 128 * tt - q0)
                            pss = psc.tile([128, 2 * QCH], F32, tag="pss")
                            for h, base in ((0, 0), (1, 64)):
                                nc.tensor.matmul(
                                    pss[:, h * QCH + r : (h + 1) * QCH],
                                    kt[t][base : base + 64, 128 * tt : 128 * (tt + 1)],
                                    qt[t][base : base + 64, q0 + r : q0 + QCH],
                                    start=True,
                                    stop=True,
                                    tile_position=(base, 0),
                                )
                            es = esp.tile([128, 2 * QCH], DT, tag="es")
                            if r == 0:
                                nc.scalar.activation(es[:], pss[:], EXP, scale=0.125)
                            else:
                                w3 = QCH - r
                                nc.scalar.activation(
                                    es.rearrange("p (h q) -> p h q", h=2)[:, :, r:QCH],
                                    pss.rearrange("p (h q) -> p h q", h=2)[:, :, r:QCH],
                                    EXP,
                                    scale=0.125,
                                )
                            if tt >= 4 * j:  # diagonal block: triangular mask
                                for h in range(2):
                                    nc.vector.tensor_tensor(
                                        out=es[:, h * QCH + r : h * QCH + r + 128],
                                        in0=es[:, h * QCH + r : h * QCH + r + 128],
                                        in1=tri[:],
                                        op=MULT,
                                    )
                            for h in range(2):
                                hcol = 65 * (2 * t + h)
                                nc.tensor.matmul(
                                    po[h][0:65, r:QCH],
                                    va[tt][:, hcol : hcol + 65],
                                    es[:, h * QCH + r : (h + 1) * QCH],
                                    start=(tt == 0),
                                    stop=(tt == last),
                                )
                        # tails: transpose back, per-partition reciprocal, store
                        for h in range(2):
                            hl = 2 * t + h
                            ot = otp.tile([128, QCH], F32, tag="ot")
                            nc.vector.tensor_copy(ot[0:65, :], po[h][0:65, :])
                            for c in range(QCH // 128):
                                pot = pso.tile([128, 65], F32, tag="po", name=f"pot{t}{j}{h}{c}")
                                nc.tensor.transpose(
                                    pot[:], ot[0:65, 128 * c : 128 * (c + 1)], idf[0:65, 0:65]
                                )
                                rc = tlp.tile([128, 1], F32, tag="rc")
                                nc.vector.reciprocal(rc[:], pot[:, 64:65])
                                on = tlp.tile([128, DH], F32, tag="on")
                                nc.vector.tensor_scalar_mul(on[:], pot[:, 0:DH], rc[:])
                                nc.sync.dma_start(
                                    out=out_d[
                                        q0 + 128 * c : q0 + 128 * (c + 1),
                                        hl * DH : (hl + 1) * DH,
                                    ],
                                    in_=on[:],
                                )

    _split_sync_waits(nc)
    return nc


def _get_nc():
    if "nc" not in _STATE:
        _STATE["nc"] = _build()
    return _STATE["nc"]


def kernel(**inputs):
    x = np.asarray(inputs["x"], dtype=np.float32)
    wq = np.asarray(inputs["Wq"], dtype=np.float32).astype(ml_dtypes.bfloat16)
    wk = np.asarray(inputs["Wk"], dtype=np.float32).astype(ml_dtypes.bfloat16)
    wv = np.asarray(inputs["Wv"], dtype=np.float32).astype(ml_dtypes.bfloat16)
    bq = np.asarray(inputs["bq"], dtype=np.float32)
    bk = np.asarray(inputs["bk"], dtype=np.float32)
    bv = np.asarray(inputs["bv"], dtype=np.float32).astype(ml_dtypes.bfloat16)
    xts = [np.ascontiguousarray(x[b].T).astype(ml_dtypes.bfloat16) for b in range(B)]

    in_maps = []
    for c in range(N_CORES):
        b, hg = divmod(c, HPC)
        sl = slice(CW * hg, CW * (hg + 1))
        in_maps.append(
            {
                "xt": xts[b],
                "wq": np.ascontiguousarray(wq[:, sl]),
                "wk": np.ascontiguousarray(wk[:, sl]),
                "wv": np.ascontiguousarray(wv[:, sl]),
                "bq": np.ascontiguousarray(bq[sl]),
                "bk": np.ascontiguousarray(bk[sl]),
                "bv": np.ascontiguousarray(bv[sl]),
            }
        )

    nc = _get_nc()
    res = run_bass_kernel_spmd(nc, in_maps, list(range(N_CORES)))
    _STATE["last_result"] = res

    out = np.empty((B, S, D), dtype=np.float32)
    for c in range(N_CORES):
        b, hg = divmod(c, HPC)
        out[b, :, CW * hg : CW * (hg + 1)] = res.results[c]["out"]
    return out


# revision 10
# speedup vs baseline: 1.0811x; 1.0251x over previous
"""Causal multi-head attention (B=2, S=2048, D=1024, H=16, Dh=64) on 8 TRN2
NeuronCores.

Sharding: core c handles batch c//4 and heads 4*(c%4) .. 4*(c%4)+3 (data
parallel on batch x tensor parallel on heads). Each core is fully
independent: it gets x[b] and the 256-wide column slices of Wq/Wk/Wv for its
4 heads, and returns its heads' outputs as a [2048, 256] slice; the host
reassembles the full [2, 2048, 1024] output by concatenation.

Device kernel (per core); matmul operands in bf16 (fp32 PSUM accumulate),
softmax normalization in fp32:
  A. x -> xT [1024,2048] via PE transposes (128x128 f32 blocks), cast to
     bf16 in the PSUM->SBUF copy.
  B. QT = Wq^T x^T + bq ([c, s] layout, one head pair per 128-partition
     tile), same for KT; V = x Wv + bv in natural [s, c] layout, stored
     augmented with a ones column per head (V_aug[:, 65h+64] = 1) so the
     attention matmul also produces the softmax denominator.
  C. Per head pair, per 512-wide q chunk, over k tiles up to the diagonal:
     scores^T[k,q] for both heads land in one 2-bank PSUM tile, one Exp
     per k tile on ScalarE (scale=1/8; no max subtraction needed, scores
     are ~N(0,1)), causal handling by column pruning + 128x128
     triangular-mask multiplies on diagonal blocks, then
     O^T[d,q] += V_aug^T @ expS accumulated in PSUM (row 64 = sum of exp).
     Tail per head: copy O^T to SBUF, PE-transpose each 128-q block back to
     [q, 65]; the denominator is then one per partition, so a [128,1]
     reciprocal + tensor_scalar multiply normalizes; DMA out in natural
     [s, e] layout.
"""

import ml_dtypes
import numpy as np

import concourse.bass as bass
import concourse.mybir as mybir
import concourse.tile as tile
from concourse.bass_utils import run_bass_kernel_spmd
from concourse.masks import make_identity, make_upper_triangular

B = 2
S = 2048
D = 1024
H = 16
DH = 64
N_CORES = 8
HPC = 4          # heads per core
CW = HPC * DH    # 256: W column slice width per core
QCH = 512        # q chunk width
F32 = mybir.dt.float32
F32R = mybir.dt.float32r
BF16 = mybir.dt.bfloat16
DT = BF16        # matmul operand dtype
EXP = mybir.ActivationFunctionType.Exp
MULT = mybir.AluOpType.mult
ADD = mybir.AluOpType.add

_STATE = {}


def _split_sync_waits(nc, max_waits=1):
    """This walrus rejects instructions carrying more than ~2 sem-waits
    ("Too many sync wait commands"). Move excess waits emitted by Tile onto
    same-engine NoOps inserted right before the instruction."""
    n = 0
    for f in nc.m.functions:
        for bb in f.blocks:
            il = bb.instructions
            i = 0
            while i < len(il):
                ins = il[i]
                si = getattr(ins, "sync_info", None)
                if si is not None and len(si.on_wait) > max_waits:
                    waits = list(si.on_wait)
                    keep = waits[len(waits) - max_waits:]
                    extra = waits[: len(waits) - max_waits]
                    ins.sync_info = mybir.SyncInfo(
                        on_wait=keep, on_update=list(si.on_update)
                    )
                    pos = i
                    for j in range(0, len(extra), max_waits):
                        nop = mybir.InstNoOp(
                            name=f"{ins.name}-waitsplit{j}",
                            engine=ins.engine,
                            sync_info=mybir.SyncInfo(
                                on_wait=extra[j : j + max_waits], on_update=[]
                            ),
                            bass_nofuse=True,
                        )
                        il.insert(pos, nop)
                        pos += 1
                        i += 1
                    n += 1
                i += 1
    return n


def _build():
    nc = bass.Bass()
    xt_d = nc.dram_tensor("xt", [D, S], BF16, kind="ExternalInput")
    wq_d = nc.dram_tensor("wq", [D, CW], BF16, kind="ExternalInput")
    wk_d = nc.dram_tensor("wk", [D, CW], BF16, kind="ExternalInput")
    wv_d = nc.dram_tensor("wv", [D, CW], BF16, kind="ExternalInput")
    bq_d = nc.dram_tensor("bq", [CW], F32, kind="ExternalInput")
    bk_d = nc.dram_tensor("bk", [CW], F32, kind="ExternalInput")
    bv_d = nc.dram_tensor("bv", [CW], BF16, kind="ExternalInput")
    out_d = nc.dram_tensor("out", [S, CW], F32, kind="ExternalOutput")

    ND = D // 128   # 8 d tiles
    NS = S // 128   # 16 s tiles
    NQ = S // QCH   # 4 q chunks

    with tile.TileContext(nc) as tc:
        with (
            tc.tile_pool(name="const", bufs=1) as cp,
            tc.tile_pool(name="big", bufs=1) as bigp,
        ):
            idf = cp.tile([128, 128], F32, tag="idf")
            tri32 = cp.tile([128, 128], F32, tag="tri32")
            ones32 = cp.tile([128, 128], F32, tag="ones32")
            tri = cp.tile([128, 128], DT, tag="tri")
            make_identity(nc, idf[:])
            make_upper_triangular(nc, tri32[:], val=1.0, diag=True)
            nc.gpsimd.memset(ones32[:], 1.0)
            nc.vector.tensor_copy(tri[:], tri32[:])

            # weights (bf16 via SWDGE cast-DMA) and biases
            wq = [bigp.tile([128, CW], DT, tag=f"wq{k}", name=f"wq{k}") for k in range(ND)]
            wk = [bigp.tile([128, CW], DT, tag=f"wk{k}", name=f"wk{k}") for k in range(ND)]
            wv = [bigp.tile([128, CW], DT, tag=f"wv{k}", name=f"wv{k}") for k in range(ND)]
            for k in range(ND):
                nc.sync.dma_start(out=wq[k][:], in_=wq_d[128 * k : 128 * (k + 1), :])
                nc.sync.dma_start(out=wk[k][:], in_=wk_d[128 * k : 128 * (k + 1), :])
                nc.sync.dma_start(out=wv[k][:], in_=wv_d[128 * k : 128 * (k + 1), :])
            bqs = cp.tile([128, 2], F32, tag="bqs")
            bks = cp.tile([128, 2], F32, tag="bks")
            bvr = cp.tile([1, CW], DT, tag="bvr")
            nc.sync.dma_start(out=bqs[:], in_=bq_d.rearrange("(t p) -> p t", p=128))
            nc.sync.dma_start(out=bks[:], in_=bk_d.rearrange("(t p) -> p t", p=128))
            nc.sync.dma_start(out=bvr[:], in_=bv_d[None, :])
            onesb = cp.tile([1, 128], DT, tag="onesb")
            nc.vector.tensor_copy(onesb[:], ones32[0:1, :])
            ones_d = cp.tile([128, HPC], DT, tag="ones_d")
            nc.vector.tensor_copy(ones_d[:], ones32[:, 0:HPC])

            xT = [bigp.tile([128, S], DT, tag=f"xT{k}", name=f"xT{k}") for k in range(ND)]
            qt = [bigp.tile([128, S], DT, tag=f"qt{t}", name=f"qt{t}") for t in range(2)]
            kt = [bigp.tile([128, S], DT, tag=f"kt{t}", name=f"kt{t}") for t in range(2)]
            va = [bigp.tile([128, 65 * HPC], DT, tag=f"va{i}", name=f"va{i}") for i in range(NS)]

            # Phase A: xT comes pre-transposed in bf16 from the host
            for k in range(ND):
                nc.sync.dma_start(out=xT[k][:], in_=xt_d[128 * k : 128 * (k + 1), :])

            # Phase B: projections
            with tc.tile_pool(name="pp", bufs=4, space="PSUM") as pp:
                for w, dstT, bsl in ((wq, qt, bqs), (wk, kt, bks)):
                    for t in range(2):
                        for j in range(NQ):
                            ppt = pp.tile([128, QCH], F32, tag="ppt")
                            for k in range(ND):
                                nc.tensor.matmul(
                                    ppt[:],
                                    w[k][:, 128 * t : 128 * (t + 1)],
                                    xT[k][:, QCH * j : QCH * (j + 1)],
                                    start=(k == 0),
                                    stop=(k == ND - 1),
                                )
                            nc.vector.tensor_scalar_add(
                                dstT[t][:, QCH * j : QCH * (j + 1)], ppt[:], bsl[:, t : t + 1]
                            )
                # bias broadcast tile for V
                ppb = pp.tile([128, CW], F32, tag="ppt")
                nc.tensor.matmul(ppb[:], onesb[0:1, :], bvr[0:1, :], start=True, stop=True)
                bcv = cp.tile([128, CW], F32, tag="bcv")
                nc.vector.tensor_copy(bcv[:], ppb[:])
                for i in range(NS):
                    ppv = pp.tile([128, CW], F32, tag="ppt")
                    for k in range(ND):
                        nc.tensor.matmul(
                            ppv[:],
                            xT[k][:, 128 * i : 128 * (i + 1)],
                            wv[k][:],
                            start=(k == 0),
                            stop=(k == ND - 1),
                        )
                    # scatter per-head 64 cols into 65-strided layout, adding bias
                    nc.vector.tensor_tensor(
                        out=va[i].rearrange("p (h e) -> p h e", h=HPC)[:, :, 0:DH],
                        in0=ppv.rearrange("p (h e) -> p h e", e=DH),
                        in1=bcv.rearrange("p (h e) -> p h e", e=DH),
                        op=ADD,
                    )
                    # ones column per head (col 65h+64)
                    nc.vector.tensor_copy(
                        va[i].rearrange("p (h e) -> p h e", h=HPC)[:, :, DH : DH + 1],
                        ones_d[:, :, None],
                    )

            # Phase C: attention; both heads of a pair share one 2-bank psS tile
            with (
                tc.tile_pool(name="esp", bufs=6) as esp,
                tc.tile_pool(name="otp", bufs=3) as otp,
                tc.tile_pool(name="tlp", bufs=4) as tlp,
                tc.tile_pool(name="psc", bufs=2, space="PSUM") as psc,
                tc.tile_pool(name="pso", bufs=4, space="PSUM") as pso,
            ):
                for t in range(2):          # head pair tile
                    for j in range(NQ):     # q chunk
                        q0 = QCH * j
                        last = 4 * j + 3
                        po = [
                            pso.tile([128, QCH], F32, tag="po", name=f"po{t}{j}{h}")
                            for h in range(2)
                        ]
                        for tt in range(4 * j + 4):   # k tiles
                            r = max(0, 128 * tt - q0)
                            pss = psc.tile([128, 2 * QCH], F32, tag="pss")
                            for h, base in ((0, 0), (1, 64)):
                                nc.tensor.matmul(
                                    pss[:, h * QCH + r : (h + 1) * QCH],
                                    kt[t][base : base + 64, 128 * tt : 128 * (tt + 1)],
                                    qt[t][base : base + 64, q0 + r : q0 + QCH],
                                    start=True,
                                    stop=True,
                                    tile_position=(base, 0),
                                )
                            es = esp.tile([128, 2 * QCH], DT, tag="es")
                            if r == 0:
                                nc.scalar.activation(es[:], pss[:], EXP, scale=0.125)
                            else:
                                w3 = QCH - r
                                nc.scalar.activation(
                                    es.rearrange("p (h q) -> p h q", h=2)[:, :, r:QCH],
                                    pss.rearrange("p (h q) -> p h q", h=2)[:, :, r:QCH],
                                    EXP,
                                    scale=0.125,
                                )
                            if tt >= 4 * j:  # diagonal block: triangular mask
                                for h in range(2):
                                    nc.vector.tensor_tensor(
                                        out=es[:, h * QCH + r : h * QCH + r + 128],
                                        in0=es[:, h * QCH + r : h * QCH + r + 128],
                                        in1=tri[:],
                                        op=MULT,
                                    )
                            for h in range(2):
                                hcol = 65 * (2 * t + h)
                                nc.tensor.matmul(
                                    po[h][0:65, r:QCH],
                                    va[tt][:, hcol : hcol + 65],
                                    es[:, h * QCH + r : (h + 1) * QCH],
                                    start=(tt == 0),
                                    stop=(tt == last),
                                )
                        # tails: transpose back, per-partition reciprocal, store
                        for h in range(2):
                            hl = 2 * t + h
                            ot = otp.tile([128, QCH], F32, tag="ot")
                            nc.vector.tensor_copy(ot[0:65, :], po[h][0:65, :])
                            for c in range(QCH // 128):
                                pot = pso.tile([128, 65], F32, tag="po", name=f"pot{t}{j}{h}{c}")
                                nc.tensor.transpose(
                                    pot[:], ot[0:65, 128 * c : 128 * (c + 1)], idf[0:65, 0:65]
                                )
                                rc = tlp.tile([128, 1], F32, tag="rc")
                                nc.vector.reciprocal(rc[:], pot[:, 64:65])
                                on = tlp.tile([128, DH], F32, tag="on")
                                nc.vector.tensor_scalar_mul(on[:], pot[:, 0:DH], rc[:])
                                nc.sync.dma_start(
                                    out=out_d[
                                        q0 + 128 * c : q0 + 128 * (c + 1),
                                        hl * DH : (hl + 1) * DH,
                                    ],
                                    in_=on[:],
                                )

    _split_sync_waits(nc)
    return nc


def _get_nc():
    if "nc" not in _STATE:
        _STATE["nc"] = _build()
    return _STATE["nc"]


def kernel(**inputs):
    x = np.asarray(inputs["x"], dtype=np.float32)
    wq = np.asarray(inputs["Wq"], dtype=np.float32).astype(ml_dtypes.bfloat16)
    wk = np.asarray(inputs["Wk"], dtype=np.float32).astype(ml_dtypes.bfloat16)
    wv = np.asarray(inputs["Wv"], dtype=np.float32).astype(ml_dtypes.bfloat16)
    bq = np.asarray(inputs["bq"], dtype=np.float32)
    bk = np.asarray(inputs["bk"], dtype=np.float32)
    bv = np.asarray(inputs["bv"], dtype=np.float32).astype(ml_dtypes.bfloat16)
    xts = [np.ascontiguousarray(x[b].T).astype(ml_dtypes.bfloat16) for b in range(B)]

    in_maps = []
    for c in range(N_CORES):
        b, hg = divmod(c, HPC)
        sl = slice(CW * hg, CW * (hg + 1))
        in_maps.append(
            {
                "xt": xts[b],
                "wq": np.ascontiguousarray(wq[:, sl]),
                "wk": np.ascontiguousarray(wk[:, sl]),
                "wv": np.ascontiguousarray(wv[:, sl]),
                "bq": np.ascontiguousarray(bq[sl]),
                "bk": np.ascontiguousarray(bk[sl]),
                "bv": np.ascontiguousarray(bv[sl]),
            }
        )

    nc = _get_nc()
    res = run_bass_kernel_spmd(nc, in_maps, list(range(N_CORES)))
    _STATE["last_result"] = res

    out = np.empty((B, S, D), dtype=np.float32)
    for c in range(N_CORES):
        b, hg = divmod(c, HPC)
        out[b, :, CW * hg : CW * (hg + 1)] = res.results[c]["out"]
    return out


# revision 13
# speedup vs baseline: 1.1649x; 1.0774x over previous
"""Causal multi-head attention (B=2, S=2048, D=1024, H=16, Dh=64) on 8 TRN2
NeuronCores.

Sharding: core c handles batch c//4 and heads 4*(c%4) .. 4*(c%4)+3 (data
parallel on batch x tensor parallel on heads). Each core is fully
independent: it gets x[b] and the 256-wide column slices of Wq/Wk/Wv for its
4 heads, and returns its heads' outputs as a [2048, 256] slice; the host
reassembles the full [2, 2048, 1024] output by concatenation.

Device kernel (per core); matmul operands in bf16 (fp32 PSUM accumulate),
softmax normalization in fp32:
  A. x -> xT [1024,2048] via PE transposes (128x128 f32 blocks), cast to
     bf16 in the PSUM->SBUF copy.
  B. QT = Wq^T x^T + bq ([c, s] layout, one head pair per 128-partition
     tile), same for KT; V = x Wv + bv in natural [s, c] layout, stored
     augmented with a ones column per head (V_aug[:, 65h+64] = 1) so the
     attention matmul also produces the softmax denominator.
  C. Per head pair, per 512-wide q chunk, over k tiles up to the diagonal:
     scores^T[k,q] for both heads land in one 2-bank PSUM tile, one Exp
     per k tile on ScalarE (scale=1/8; no max subtraction needed, scores
     are ~N(0,1)), causal handling by column pruning + 128x128
     triangular-mask multiplies on diagonal blocks, then
     O^T[d,q] += V_aug^T @ expS accumulated in PSUM (row 64 = sum of exp).
     Tail per head: copy O^T to SBUF, PE-transpose each 128-q block back to
     [q, 65]; the denominator is then one per partition, so a [128,1]
     reciprocal + tensor_scalar multiply normalizes; DMA out in natural
     [s, e] layout.
"""

import ml_dtypes
import numpy as np

import concourse.bass as bass
import concourse.mybir as mybir
import concourse.tile as tile
from concourse.bass_utils import run_bass_kernel_spmd
from concourse.masks import make_identity, make_upper_triangular

B = 2
S = 2048
D = 1024
H = 16
DH = 64
N_CORES = 8
HPC = 4          # heads per core
CW = HPC * DH    # 256: W column slice width per core
QCH = 512        # q chunk width
F32 = mybir.dt.float32
F32R = mybir.dt.float32r
BF16 = mybir.dt.bfloat16
DT = BF16        # matmul operand dtype
EXP = mybir.ActivationFunctionType.Exp
MULT = mybir.AluOpType.mult
ADD = mybir.AluOpType.add

_STATE = {}


def _split_sync_waits(nc, max_waits=1):
    """This walrus rejects instructions carrying more than ~2 sem-waits
    ("Too many sync wait commands"). Move excess waits emitted by Tile onto
    same-engine NoOps inserted right before the instruction."""
    n = 0
    for f in nc.m.functions:
        for bb in f.blocks:
            il = bb.instructions
            i = 0
            while i < len(il):
                ins = il[i]
                si = getattr(ins, "sync_info", None)
                if si is not None and len(si.on_wait) > max_waits:
                    waits = list(si.on_wait)
                    keep = waits[len(waits) - max_waits:]
                    extra = waits[: len(waits) - max_waits]
                    ins.sync_info = mybir.SyncInfo(
                        on_wait=keep, on_update=list(si.on_update)
                    )
                    pos = i
                    for j in range(0, len(extra), max_waits):
                        nop = mybir.InstNoOp(
                            name=f"{ins.name}-waitsplit{j}",
                            engine=ins.engine,
                            sync_info=mybir.SyncInfo(
                                on_wait=extra[j : j + max_waits], on_update=[]
                            ),
                            bass_nofuse=True,
                        )
                        il.insert(pos, nop)
                        pos += 1
                        i += 1
                    n += 1
                i += 1
    return n


def _build():
    nc = bass.Bass()
    xt_d = nc.dram_tensor("xt", [D, S], BF16, kind="ExternalInput")
    wq_d = nc.dram_tensor("wq", [D, CW], BF16, kind="ExternalInput")
    wk_d = nc.dram_tensor("wk", [D, CW], BF16, kind="ExternalInput")
    wv_d = nc.dram_tensor("wv", [D, CW], BF16, kind="ExternalInput")
    bq_d = nc.dram_tensor("bq", [CW], F32, kind="ExternalInput")
    bk_d = nc.dram_tensor("bk", [CW], F32, kind="ExternalInput")
    bv_d = nc.dram_tensor("bv", [CW], BF16, kind="ExternalInput")
    out_d = nc.dram_tensor("out", [S, CW], F32, kind="ExternalOutput")

    ND = D // 128   # 8 d tiles
    NS = S // 128   # 16 s tiles
    NQ = S // QCH   # 4 q chunks

    with tile.TileContext(nc) as tc:
        with (
            tc.tile_pool(name="const", bufs=1) as cp,
            tc.tile_pool(name="big", bufs=1) as bigp,
        ):
            idf = cp.tile([128, 128], F32, tag="idf")
            tri32 = cp.tile([128, 128], F32, tag="tri32")
            ones32 = cp.tile([128, 128], F32, tag="ones32")
            tri = cp.tile([128, 128], DT, tag="tri")
            make_identity(nc, idf[:])
            make_upper_triangular(nc, tri32[:], val=1.0, diag=True)
            nc.gpsimd.memset(ones32[:], 1.0)
            nc.vector.tensor_copy(tri[:], tri32[:])

            # weights (bf16, one DMA per matrix) and biases
            wqall = bigp.tile([128, ND * CW], DT, tag="wqall")
            wkall = bigp.tile([128, ND * CW], DT, tag="wkall")
            wvall = bigp.tile([128, ND * CW], DT, tag="wvall")
            nc.sync.dma_start(out=wqall.rearrange("p (k c) -> p k c", c=CW), in_=wq_d.rearrange("(k p) c -> p k c", p=128))
            nc.sync.dma_start(out=wkall.rearrange("p (k c) -> p k c", c=CW), in_=wk_d.rearrange("(k p) c -> p k c", p=128))
            nc.sync.dma_start(out=wvall.rearrange("p (k c) -> p k c", c=CW), in_=wv_d.rearrange("(k p) c -> p k c", p=128))
            wq = [wqall[:, CW * k : CW * (k + 1)] for k in range(ND)]
            wk = [wkall[:, CW * k : CW * (k + 1)] for k in range(ND)]
            wv = [wvall[:, CW * k : CW * (k + 1)] for k in range(ND)]
            bqs = cp.tile([128, 2], F32, tag="bqs")
            bks = cp.tile([128, 2], F32, tag="bks")
            bvr = cp.tile([1, CW], DT, tag="bvr")
            nc.sync.dma_start(out=bqs[:], in_=bq_d.rearrange("(t p) -> p t", p=128))
            nc.sync.dma_start(out=bks[:], in_=bk_d.rearrange("(t p) -> p t", p=128))
            nc.sync.dma_start(out=bvr[:], in_=bv_d[None, :])
            onesb = cp.tile([1, 128], DT, tag="onesb")
            nc.vector.tensor_copy(onesb[:], ones32[0:1, :])
            ones_d = cp.tile([128, HPC], DT, tag="ones_d")
            nc.vector.tensor_copy(ones_d[:], ones32[:, 0:HPC])

            xTall = bigp.tile([128, ND * S], DT, tag="xTall")
            xT = [xTall[:, S * k : S * (k + 1)] for k in range(ND)]
            qt = [bigp.tile([128, S], DT, tag=f"qt{t}", name=f"qt{t}") for t in range(2)]
            kt = [bigp.tile([128, S], DT, tag=f"kt{t}", name=f"kt{t}") for t in range(2)]
            va = [bigp.tile([128, 65 * HPC], DT, tag=f"va{i}", name=f"va{i}") for i in range(NS)]

            # Phase A: xT comes pre-transposed in bf16 from the host, one DMA
            nc.sync.dma_start(out=xTall.rearrange("p (k s) -> p k s", s=S), in_=xt_d.rearrange("(k p) s -> p k s", p=128))

            # Phase B: projections
            with tc.tile_pool(name="pp", bufs=4, space="PSUM") as pp:
                for w, dstT, bsl in ((wq, qt, bqs), (wk, kt, bks)):
                    for t in range(2):
                        for j in range(NQ):
                            ppt = pp.tile([128, QCH], F32, tag="ppt")
                            for k in range(ND):
                                nc.tensor.matmul(
                                    ppt[:],
                                    w[k][:, 128 * t : 128 * (t + 1)],
                                    xT[k][:, QCH * j : QCH * (j + 1)],
                                    start=(k == 0),
                                    stop=(k == ND - 1),
                                )
                            nc.vector.tensor_scalar_add(
                                dstT[t][:, QCH * j : QCH * (j + 1)], ppt[:], bsl[:, t : t + 1]
                            )
                # bias broadcast tile for V
                ppb = pp.tile([128, CW], F32, tag="ppt")
                nc.tensor.matmul(ppb[:], onesb[0:1, :], bvr[0:1, :], start=True, stop=True)
                bcv = cp.tile([128, CW], F32, tag="bcv")
                nc.vector.tensor_copy(bcv[:], ppb[:])
                for i in range(NS):
                    ppv = pp.tile([128, CW], F32, tag="ppt")
                    for k in range(ND):
                        nc.tensor.matmul(
                            ppv[:],
                            xT[k][:, 128 * i : 128 * (i + 1)],
                            wv[k],
                            start=(k == 0),
                            stop=(k == ND - 1),
                        )
                    # scatter per-head 64 cols into 65-strided layout, adding bias
                    nc.vector.tensor_tensor(
                        out=va[i].rearrange("p (h e) -> p h e", h=HPC)[:, :, 0:DH],
                        in0=ppv.rearrange("p (h e) -> p h e", e=DH),
                        in1=bcv.rearrange("p (h e) -> p h e", e=DH),
                        op=ADD,
                    )
                    # ones column per head (col 65h+64)
                    nc.vector.tensor_copy(
                        va[i].rearrange("p (h e) -> p h e", h=HPC)[:, :, DH : DH + 1],
                        ones_d[:, :, None],
                    )

            # Phase C: attention; j outer so all 4 heads' tails of a q chunk
            # merge into one [128, 256] output tile per 128-row block (1 DMA).
            with (
                tc.tile_pool(name="esp", bufs=4) as esp,
                tc.tile_pool(name="otp", bufs=5) as otp,
                tc.tile_pool(name="tlp", bufs=4) as tlp,
                tc.tile_pool(name="onp", bufs=3) as onp,
                tc.tile_pool(name="psc", bufs=2, space="PSUM") as psc,
                tc.tile_pool(name="pso", bufs=4, space="PSUM") as pso,
            ):
                for j in range(NQ):     # q chunk
                    q0 = QCH * j
                    last = 4 * j + 3
                    ots = {}
                    for t in range(2):  # head pair tile
                        po = [
                            pso.tile([128, QCH], F32, tag="po", name=f"po{t}{j}{h}")
                            for h in range(2)
                        ]
                        for tt in range(4 * j + 4):   # k tiles
                            r = max(0, 128 * tt - q0)
                            pss = psc.tile([128, 2 * QCH], F32, tag="pss")
                            for h, base in ((0, 0), (1, 64)):
                                nc.tensor.matmul(
                                    pss[:, h * QCH + r : (h + 1) * QCH],
                                    kt[t][base : base + 64, 128 * tt : 128 * (tt + 1)],
                                    qt[t][base : base + 64, q0 + r : q0 + QCH],
                                    start=True,
                                    stop=True,
                                    tile_position=(base, 0),
                                )
                            es = esp.tile([128, 2 * QCH], DT, tag="es")
                            if r == 0:
                                nc.scalar.activation(es[:], pss[:], EXP, scale=0.125)
                            else:
                                nc.scalar.activation(
                                    es.rearrange("p (h q) -> p h q", h=2)[:, :, r:QCH],
                                    pss.rearrange("p (h q) -> p h q", h=2)[:, :, r:QCH],
                                    EXP,
                                    scale=0.125,
                                )
                            if tt >= 4 * j:  # diagonal block: triangular mask
                                for h in range(2):
                                    nc.vector.tensor_tensor(
                                        out=es[:, h * QCH + r : h * QCH + r + 128],
                                        in0=es[:, h * QCH + r : h * QCH + r + 128],
                                        in1=tri[:],
                                        op=MULT,
                                    )
                            for h in range(2):
                                hcol = 65 * (2 * t + h)
                                nc.tensor.matmul(
                                    po[h][0:65, r:QCH],
                                    va[tt][:, hcol : hcol + 65],
                                    es[:, h * QCH + r : (h + 1) * QCH],
                                    start=(tt == 0),
                                    stop=(tt == last),
                                )
                        for h in range(2):
                            ot = otp.tile([128, QCH], F32, tag="ot")
                            nc.vector.tensor_copy(ot[0:65, :], po[h][0:65, :])
                            ots[(t, h)] = ot
                    # tails: transpose back, per-partition reciprocal; all four
                    # heads of this q block share one output tile and one DMA
                    for c in range(QCH // 128):
                        on = onp.tile([128, CW], F32, tag="on")
                        for t in range(2):
                            for h in range(2):
                                hl = 2 * t + h
                                pot = pso.tile(
                                    [128, 65], F32, tag="po", name=f"pot{j}{c}{t}{h}"
                                )
                                nc.tensor.transpose(
                                    pot[:],
                                    ots[(t, h)][0:65, 128 * c : 128 * (c + 1)],
                                    idf[0:65, 0:65],
                                )
                                rc = tlp.tile([128, 1], F32, tag="rc")
                                nc.vector.reciprocal(rc[:], pot[:, 64:65])
                                nc.vector.tensor_scalar_mul(
                                    on[:, hl * DH : (hl + 1) * DH], pot[:, 0:DH], rc[:]
                                )
                        nc.sync.dma_start(
                            out=out_d[q0 + 128 * c : q0 + 128 * (c + 1), :], in_=on[:]
                        )

    _split_sync_waits(nc)
    return nc


def _get_nc():
    if "nc" not in _STATE:
        _STATE["nc"] = _build()
    return _STATE["nc"]


def kernel(**inputs):
    x = np.asarray(inputs["x"], dtype=np.float32)
    wq = np.asarray(inputs["Wq"], dtype=np.float32).astype(ml_dtypes.bfloat16)
    wk = np.asarray(inputs["Wk"], dtype=np.float32).astype(ml_dtypes.bfloat16)
    wv = np.asarray(inputs["Wv"], dtype=np.float32).astype(ml_dtypes.bfloat16)
    bq = np.asarray(inputs["bq"], dtype=np.float32)
    bk = np.asarray(inputs["bk"], dtype=np.float32)
    bv = np.asarray(inputs["bv"], dtype=np.float32).astype(ml_dtypes.bfloat16)
    xts = [np.ascontiguousarray(x[b].T).astype(ml_dtypes.bfloat16) for b in range(B)]

    in_maps = []
    for c in range(N_CORES):
        b, hg = divmod(c, HPC)
        sl = slice(CW * hg, CW * (hg + 1))
        in_maps.append(
            {
                "xt": xts[b],
                "wq": np.ascontiguousarray(wq[:, sl]),
                "wk": np.ascontiguousarray(wk[:, sl]),
                "wv": np.ascontiguousarray(wv[:, sl]),
                "bq": np.ascontiguousarray(bq[sl]),
                "bk": np.ascontiguousarray(bk[sl]),
                "bv": np.ascontiguousarray(bv[sl]),
            }
        )

    nc = _get_nc()
    res = run_bass_kernel_spmd(nc, in_maps, list(range(N_CORES)))
    _STATE["last_result"] = res

    out = np.empty((B, S, D), dtype=np.float32)
    for c in range(N_CORES):
        b, hg = divmod(c, HPC)
        out[b, :, CW * hg : CW * (hg + 1)] = res.results[c]["out"]
    return out
